# revision 1
# baseline (speedup 1.0000x reference)
"""ACmix (local 3x3 window attention + dynamic conv mix) on 8 TRN2 NeuronCores.

Sharding: data-parallel over batch B=8, one batch element per core.
Per-core layout: channels (128) on partitions, L = H*W = 4096 on the free dim.

v2: quarter-granular pipeline.
  qkv   = w_qkv @ x                       (PE; PSUM in 512-chunks, scalar copies)
  per quarter qt (1024 cols = 16 image rows):
    pr_s = q * k_pad-shift_s              (DVE, 9 shifts)
    logits = sum_s bones_s @ pr_s         (PE, per 512-chunk, PSUM rows 0:36)
    e = exp(logits)                       (scalar)
    sums = sones2 @ e                     (PE, SAME psum bank rows 64:100)
    r = 1/sums                            (DVE reciprocal, f32)
    e *= r ; c = kern + e                 (DVE)  [kern = wkb matmuls + bias]
    c -> DRAM -> 9x replicate-read DMA -> c_exp_s [128 rows]
    p2_s = c_exp_s * v_pad-shift_s        (DVE for 6 shifts, GpSimd for 3)
    out_psum += w_proj @ p2_s             (PE, folded projection, 9-matmul groups)
"""

import os
from contextlib import ExitStack

import numpy as np
import ml_dtypes

import concourse.bass as bass
import concourse.bacc as bacc
import concourse.tile as tile
from concourse import mybir
from concourse.bass_utils import run_bass_kernel_spmd

BF16 = mybir.dt.bfloat16
F32 = mybir.dt.float32
NPBF16 = ml_dtypes.bfloat16

B, C, H, W = 8, 128, 64, 64
L = H * W                      # 4096
NH, HD, K2 = 4, 32, 9
PW, PH = W + 2, H + 2          # 66
PL = PW * PH                   # 4356
SHIFTS = [(di, dj) for di in (-1, 0, 1) for dj in (-1, 0, 1)]  # k2 = 3(di+1)+(dj+1)
NQ = 4                         # quarters (1024 cols = 16 image rows each)

# weight blob column layout (bf16, 128 partitions) — same as v1 blob
OFF_QKV = 0            # wqkvT [128, 384]
OFF_PROJ = 384         # wprojT [128, 128]
OFF_WKB = 512          # wkbT 3 x [128, 36]
OFF_S2 = 620           # sones2 [36, 36] (rows 36.. zero)
OFF_BONES = 656        # bones 9 x [128, 36]
OFF_EXP = 656 + 9 * 36  # 980: E_s [36 rows, 128] x PE-shifts (unused in v2)
PE_SHIFTS = [3, 5, 8]
WBLOB = 980 + len(PE_SHIFTS) * 128

GP_SHIFTS = [0, 4, 8]          # stage-2 product shifts routed to GpSimd

TRACE = False
LAST_RESULTS = None


def _ensure_profile_hook():
    """Provide antenv.axon_hooks (missing in this container's antenv stub)
    so run_bass_kernel_spmd(trace=True) can capture NTFF profiles."""
    import sys, types
    try:
        from antenv.axon_hooks import get_axon_ntff_profile_hook  # noqa: F401
        return
    except ImportError:
        pass
    try:
        from trn_agent_boot.trn_boot import _ntff_profile_via_ctypes
        hook = _ntff_profile_via_ctypes("/opt/axon/libaxon_pjrt.so")
    except Exception:
        hook = None
    mod = types.ModuleType("antenv.axon_hooks")
    state = {"hook": hook}
    mod.get_axon_ntff_profile_hook = lambda: state["hook"]
    mod.set_axon_ntff_profile_hook = lambda h: state.__setitem__("hook", h)
    sys.modules["antenv.axon_hooks"] = mod
    import antenv
    antenv.axon_hooks = mod


def _build(subtract_m: bool):
    nc = bacc.Bacc("TRN2", target_bir_lowering=False, debug=False)
    x_ext = nc.declare_dram_parameter("x", [C, L], BF16, isOutput=False)
    wblob_ext = nc.declare_dram_parameter("wblob", [C, WBLOB], BF16, isOutput=False)
    bias_ext = nc.declare_dram_parameter("bias", [36, 1], F32, isOutput=False)
    out_ext = nc.declare_dram_parameter("out", [C, L], BF16, isOutput=True)

    with tile.TileContext(nc) as tc, ExitStack() as ctx:
        pw = ctx.enter_context(tc.tile_pool(name="weights", bufs=1))
        pmain = ctx.enter_context(tc.tile_pool(name="main", bufs=1))
        psmall = ctx.enter_context(tc.tile_pool(name="small", bufs=2))
        pprod = ctx.enter_context(tc.tile_pool(name="prods", bufs=2))
        pdram = ctx.enter_context(tc.tile_pool(name="dram", bufs=1, space="DRAM"))
        c_dram = pdram.tile([4 * 36, L], BF16, name="c_rt")  # 4 per-head copies
        c_ap = c_dram[:]

        # ---- input / weight DMAs (x quarter 0 first for fastest start) ----
        x_sb = pmain.tile([C, L], BF16)
        wblob = pw.tile([C, WBLOB], BF16)
        nc.sync.dma_start(x_sb[:, 0:1024], x_ext[:, 0:1024])
        nc.sync.dma_start(wblob[:, 0:384], wblob_ext[:, 0:384])
        for xq in range(1, 4):
            nc.sync.dma_start(x_sb[:, xq * 1024 : xq * 1024 + 1024],
                              x_ext[:, xq * 1024 : xq * 1024 + 1024])
        wq = (WBLOB - 384 + 2) // 3
        for wi in range(3):
            a, b = 384 + wi * wq, min(WBLOB, 384 + wi * wq + wq)
            nc.sync.dma_start(wblob[:, a:b], wblob_ext[:, a:b])
        bias_sb = pw.tile([36, 1], F32)
        nc.sync.dma_start(bias_sb[:], bias_ext[:])

        wqkvT = wblob[:, OFF_QKV : OFF_QKV + 384]
        wprojT = wblob[:, OFF_PROJ : OFF_PROJ + 128]
        wkbT = [wblob[:, OFF_WKB + 36 * j : OFF_WKB + 36 * j + 36] for j in range(3)]
        sones2 = wblob[0:36, OFF_S2 : OFF_S2 + 36]
        bones = [wblob[:, OFF_BONES + 36 * s : OFF_BONES + 36 * s + 36] for s in range(9)]

        q_sb = pmain.tile([C, L], BF16)
        k_pad = pmain.tile([C, PL], BF16)
        v_pad = pmain.tile([C, PL], BF16)
        for t in (k_pad, v_pad):
            t3 = t[:].rearrange("p (r c) -> p r c", c=PW)
            nc.gpsimd.memset(t3[:, 0, :], 0.0)          # top padded row
            nc.gpsimd.memset(t3[:, PH - 1, :], 0.0)     # bottom padded row
            nc.gpsimd.memset(t3[:, 1 : PH - 1, 0:1], 0.0)
            nc.gpsimd.memset(t3[:, 1 : PH - 1, PW - 1 : PW], 0.0)

        kp3 = k_pad[:].rearrange("p (r c) -> p r c", c=PW)
        vp3 = v_pad[:].rearrange("p (r c) -> p r c", c=PW)

        out_sb = pmain.tile([C, L], BF16)

        psB = ctx.enter_context(tc.tile_pool(name="psB", bufs=2, space="PSUM"))
        psK = ctx.enter_context(tc.tile_pool(name="psK", bufs=2, space="PSUM"))

        # ---- Phase A: qkv = w_qkv @ x (psQ scoped: banks recycled to psC) --
        # order: k h0, k h1, q h0, v h0 | front(0) | q h1, v h1
        psQ_ctx = tc.tile_pool(name="psQ", bufs=2, space="PSUM")
        psQ = psQ_ctx.__enter__()
        if True:
            def qkv_part(t, hf):
                dst3 = (k_pad if t == 1 else v_pad)[:].rearrange(
                    "p (r c) -> p r c", c=PW) if t != 0 else None
                for mc in range(4):
                    col = hf * 2048 + mc * 512
                    ps = psQ.tile([C, 512], F32, tag="qkv", name=f"qkv{t}_{hf}_{mc}")
                    nc.tensor.matmul(ps[:], wqkvT[:, t * C : t * C + C],
                                     x_sb[:, col : col + 512], start=True, stop=True)
                    if t == 0:
                        nc.scalar.copy(q_sb[:, col : col + 512], ps[:])
                    else:
                        r0 = hf * 32 + mc * 8
                        nc.scalar.copy(
                            dst3[:, 1 + r0 : 1 + r0 + 8, 1 : 1 + W],
                            ps[:].rearrange("p (r c) -> p r c", c=W),
                        )

            qkv_rest = [(0, 1), (2, 1)]

            for t, hf in ((1, 0), (1, 1), (0, 0), (2, 0)):
                qkv_part(t, hf)

        # ---- Phase B/C fused, per quarter ---------------------------------
        def products(qt):
            r0q = qt * 16
            prodmap = {}
            for s in range(9):
                di, dj = SHIFTS[s]
                pr = pprod.tile([C, 1024], BF16, tag=f"pr{s}", name=f"pr{s}_{qt}")
                psrc = kp3[:, 1 + di + r0q : 1 + di + r0q + 16, 1 + dj : 1 + dj + W]
                nc.vector.tensor_mul(
                    pr[:].rearrange("p (r c) -> p r c", c=W),
                    q_sb[:, qt * 1024 : qt * 1024 + 1024].rearrange(
                        "p (r c) -> p r c", c=W),
                    psrc,
                )
                prodmap[s] = pr
            return prodmap

        def sums_mm(qt, sub, e_q, sm_qs):
            sm = psB.tile([36, 512], F32, tag="sm", name=f"sm{qt}_{sub}")
            nc.tensor.matmul(
                sm[:], sones2, e_q[:, sub * 512 : sub * 512 + 512],
                start=True, stop=True, skip_group_check=True,
            )
            sm_qs[(qt, sub)] = sm

        def bones_exp(qt, prodmap, e_q):
            # PE: logits per 512-chunk -> scalar exp
            for sub in range(2):
                lg = psB.tile([36, 512], F32, tag="lg", name=f"lg{qt}_{sub}")
                for s in range(9):
                    nc.tensor.matmul(
                        lg[:], bones[s],
                        prodmap[s][:, sub * 512 : sub * 512 + 512],
                        start=(s == 0), stop=(s == 8),
                    )
                nc.scalar.activation(
                    e_q[:, sub * 512 : sub * 512 + 512], lg[:],
                    mybir.ActivationFunctionType.Exp,
                )

        def kern_quarter(qt, kern_q):
            r0q = qt * 16
            for sub in range(2):
                psk = psK.tile([36, 512], F32, tag="kern", name=f"kern{qt}_{sub}")
                col = qt * 1024 + sub * 512
                rr = r0q + sub * 8
                nc.tensor.matmul(psk[:], wkbT[0], q_sb[:, col : col + 512],
                                 start=True, stop=False)
                nc.tensor.matmul(psk[:], wkbT[1],
                                 kp3[:, 1 + rr : 1 + rr + 8, 1 : 1 + W],
                                 start=False, stop=False)
                nc.tensor.matmul(psk[:], wkbT[2],
                                 vp3[:, 1 + rr : 1 + rr + 8, 1 : 1 + W],
                                 start=False, stop=True)
                nc.scalar.activation(
                    kern_q[:, sub * 512 : sub * 512 + 512], psk[:],
                    mybir.ActivationFunctionType.Identity, bias=bias_sb[:],
                )

        def tail(qt, e_q, kern_q, c_q, sm_qs):
            # DVE: recip + e*r + c = kern + e (sums already on PE)
            if subtract_m:
                nc.vector.tensor_copy(c_q[:], kern_q[:])
            else:
                r_q = psmall.tile([36, 1024], F32, tag="r", name=f"r{qt}")
                for sub in range(2):
                    nc.vector.reciprocal_approx_fast(
                        r_q[:, sub * 512 : sub * 512 + 512], sm_qs[(qt, sub)][:])
                nc.vector.tensor_mul(e_q[:], e_q[:], r_q[:])
                nc.vector.tensor_add(c_q[:], kern_q[:], e_q[:])
            # write 4 DRAM copies (one per head) so expansion reads stripe
            # across distinct DRAM regions: dst iterates (row, copy, col)
            nc.sync.dma_start(
                bass.AP(c_ap.tensor, c_ap.offset + qt * 1024,
                        [[L, 36], [36 * L, 4], [1, 1024]]),
                bass.AP(c_q.tensor, c_q.offset, [[c_q.ap[0][0], 36], [0, 4], [1, 1024]]),
            )

        cexp_qs = {}

        def expansion_dma(qt):
            # one batched replicate-read: [4 heads, 32 reps, 9 shifts, 1024 cols]
            ce = pmain.tile([C, 9 * 1024], BF16, tag="cexpq", bufs=2,
                            name=f"cexpq{qt}")
            cexp_qs[qt] = ce
            for h in range(4):
                src_ap = bass.AP(
                    c_ap.tensor, c_ap.offset + (36 * h + 9 * h) * L + qt * 1024,
                    [[0, 32], [L, 9], [1, 1024]],
                )
                nc.sync.dma_start(ce[32 * h : 32 * h + 32, :], src_ap)

        outps_qs = {}
        p2map_qs = {}

        def p2_mul(qt, s, eng):
            r0q = qt * 16
            di, dj = SHIFTS[s]
            vsrc = vp3[:, 1 + di + r0q : 1 + di + r0q + 16, 1 + dj : 1 + dj + W]
            p2 = pprod.tile([C, 1024], BF16, tag=f"p2_{s}", name=f"p2_{qt}_{s}")
            p2map_qs[qt][s] = p2
            eng.tensor_mul(
                p2[:].rearrange("p (r c) -> p r c", c=W),
                cexp_qs[qt][:, s * 1024 : s * 1024 + 1024].rearrange(
                    "p (r c) -> p r c", c=W),
                vsrc,
            )

        def stage2_gp(qt):
            # slow gpsimd products issued early, right behind the expansion DMA
            p2map_qs[qt] = {}
            for s in GP_SHIFTS:
                p2_mul(qt, s, nc.gpsimd)

        def stage2(qt):
            outps = [psC.tile([C, 512], F32, tag=f"out{sub}", name=f"outps{qt}_{sub}")
                     for sub in range(2)]
            outps_qs[qt] = outps
            dve_shifts = [s for s in range(9) if s not in GP_SHIFTS]
            for s in dve_shifts:
                p2_mul(qt, s, nc.vector)
            # proj accumulation: gp shifts first so the stop lands on fast DVE p2
            mm_order = list(GP_SHIFTS) + dve_shifts
            for si, s in enumerate(mm_order):
                for sub in range(2):
                    nc.tensor.matmul(
                        outps[sub][:], wprojT,
                        p2map_qs[qt][s][:, sub * 512 : sub * 512 + 512],
                        start=(si == 0), stop=(si == 8),
                        skip_group_check=True,
                    )

        def stage2_out(qt):
            for sub in range(2):
                ci = qt * 2 + sub
                nc.scalar.copy(out_sb[:, ci * 512 : ci * 512 + 512],
                               outps_qs[qt][sub][:])
            nc.sync.dma_start(
                out_ext[:, qt * 1024 : qt * 1024 + 1024],
                out_sb[:, qt * 1024 : qt * 1024 + 1024],
            )

        # pipeline with 1-quarter skew between produce and consume stages;
        # PE per quarter: bones x18, sums(s0), wkb x6, sums(s1)
        e_qs, kern_qs, c_qs, prod_qs, sm_qs = {}, {}, {}, {}, {}
        for qt in range(NQ):
            e_qs[qt] = psmall.tile([36, 1024], BF16, tag="e", name=f"e{qt}")
            kern_qs[qt] = psmall.tile([36, 1024], BF16, tag="kern", name=f"kern_q{qt}")
            c_qs[qt] = psmall.tile([36, 1024], BF16, tag="c", name=f"c_q{qt}")

        def quarter_front(qt):
            prod_qs[qt] = products(qt)
            bones_exp(qt, prod_qs[qt], e_qs[qt])
            sums_mm(qt, 0, e_qs[qt], sm_qs)
            kern_quarter(qt, kern_qs[qt])
            sums_mm(qt, 1, e_qs[qt], sm_qs)

        def quarter_tail(qt):
            tail(qt, e_qs[qt], kern_qs[qt], c_qs[qt], sm_qs)
            expansion_dma(qt)

        quarter_front(0)
        for t, hf in qkv_rest:
            qkv_part(t, hf)
        psQ_ctx.__exit__(None, None, None)
        psC = ctx.enter_context(tc.tile_pool(name="psC", bufs=1, space="PSUM"))
        quarter_front(1)
        quarter_tail(0)
        stage2_gp(0)
        quarter_front(2)
        quarter_tail(1)
        stage2_gp(1)
        stage2(0)
        quarter_front(3)
        quarter_tail(2)
        stage2_gp(2)
        stage2(1)
        stage2_out(0)
        quarter_tail(3)
        stage2_gp(3)
        stage2(2)
        stage2_out(1)
        stage2(3)
        stage2_out(2)
        stage2_out(3)

    nc.compile()
    return nc


_GRAPH_CACHE = {}


def _get_graph(subtract_m: bool):
    if subtract_m not in _GRAPH_CACHE:
        _GRAPH_CACHE[subtract_m] = _build(subtract_m)
    return _GRAPH_CACHE[subtract_m]


def prepare_feeds(x, w_qkv, w_kernel, b_kernel, w_proj, alpha, beta):
    x = np.asarray(x, np.float32)
    w_qkv = np.asarray(w_qkv, np.float32)
    w_kernel = np.asarray(w_kernel, np.float32)
    b_kernel = np.asarray(b_kernel, np.float32)
    w_proj = np.asarray(w_proj, np.float32)
    alpha = float(np.asarray(alpha))
    beta = float(np.asarray(beta))

    # Fold alpha into the output projection and beta/alpha into the kernel
    # branch so the attention coefficient is exactly e/sums on device.
    alpha0 = (alpha == 0.0)
    if alpha0:
        proj_scale, kb_scale = 1.0, beta
    else:
        proj_scale, kb_scale = alpha, beta / alpha

    xb = x.reshape(B, C, L).astype(NPBF16)
    blob = np.zeros((C, WBLOB), np.float32)
    blob[:, OFF_QKV : OFF_QKV + 384] = w_qkv.T
    blob[:, OFF_PROJ : OFF_PROJ + 128] = proj_scale * w_proj.T
    wkb_full = np.zeros((36, 3 * C), np.float32)
    for g in range(4):
        wkb_full[g * 9 : g * 9 + 9, 96 * g : 96 * g + 96] = kb_scale * w_kernel[g * 9 : g * 9 + 9]
    for j in range(3):
        blob[:, OFF_WKB + 36 * j : OFF_WKB + 36 * j + 36] = wkb_full[:, 128 * j : 128 * j + 128].T
    for h in range(4):
        blob[9 * h : 9 * h + 9, OFF_S2 + 9 * h : OFF_S2 + 9 * h + 9] = 1.0
    for s in range(9):
        for d in range(C):
            blob[d, OFF_BONES + 36 * s + 9 * (d // 32) + s] = 1.0
    for i, s in enumerate(PE_SHIFTS):
        for d in range(C):
            blob[9 * (d // 32) + s, OFF_EXP + 128 * i + d] = 1.0
    blob = blob.astype(NPBF16)
    bias = (kb_scale * b_kernel).reshape(36, 1).astype(np.float32)
    feeds = [
        {"x": np.ascontiguousarray(xb[b]), "wblob": blob, "bias": bias}
        for b in range(B)
    ]
    return feeds, alpha0


def kernel(x, w_qkv, w_kernel, b_kernel, w_proj, alpha, beta):
    global LAST_RESULTS
    in_maps, subtract_m = prepare_feeds(x, w_qkv, w_kernel, b_kernel, w_proj, alpha, beta)
    nc = _get_graph(subtract_m)
    if TRACE:
        _ensure_profile_hook()
    res = run_bass_kernel_spmd(nc, in_maps, list(range(B)), trace=TRACE)
    LAST_RESULTS = res
    out = np.stack([np.asarray(res.results[b]["out"], np.float32) for b in range(B)])
    return out.reshape(B, C, H, W)

